# revision 8
# baseline (speedup 1.0000x reference)
"""AttnReweight kernel for Trainium2 (8 NeuronCores, SPMD data parallel).

Semantics (matching the reference):
    c = max(attn); a = exp(attn - c)
    pj[b,s,h,w,k] = sum_t sims[b,hj,wj,t] * (sinds[b,hj,wj,t] == sinds[b,h,w,s])
                    where (hj,wj) = clamped 3x3 neighbor k of (h,w)
    m = a[b,d,h,w,k] * pj[b,s,h,w,k]
    out[b,d,s,h,w,k] = m / (1e-10 + sum_k m)

Sharding: core = b*4 + q handles image b, rows [48q, 48q+48), all heads d and
all slots s.  Pure SPMD, no collectives; halos and row-shifted copies are
materialized host-side so the program is identical on every core.

Engine split: exact fp32 match (is_equal / mult / add) on VectorE; per-head
multiplies on GpSimd; exp and the 1/(den+eps) (computed as exp(-ln(den+eps)))
on ScalarE.  The image is processed in two w-halves so the per-head stage of
half A overlaps the match of half B.

On-chip layout: 96 partitions = 2 w-halves x 48 rows.
"""

import numpy as np

B, HD, H, W, K, NSP = 2, 8, 192, 192, 9, 9
NCORES = 8
ROWS = 48            # image rows per core
G = 2                # w segments per core
WSEG = 96            # interior w positions per segment
WSPAN = WSEG + 2     # with w halo
PI = G * ROWS        # 96 partitions
FR = WSPAN * 9       # 882: padded (w, slot) tiles
NWH = 2              # w-half split for phase overlap
WH = WSEG // NWH     # 48 w per half
FH = WH * NSP        # 432: match op free size (w-half, s)
FD = NSP * WH * K    # 3888: per-(d, half) work size (s, w, k)
EPS = 1e-10
OFFS = [(dh, dw) for dh in (-1, 0, 1) for dw in (-1, 0, 1)]

_compiled = None


def _build():
    from contextlib import ExitStack

    import concourse.bacc as bacc
    import concourse.tile as tile
    from concourse import mybir

    f32 = mybir.dt.float32
    Alu = mybir.AluOpType
    Act = mybir.ActivationFunctionType

    nc = bacc.Bacc(
        "TRN2",
        target_bir_lowering=False,
        debug=False,
        enable_asserts=True,
        num_devices=NCORES,
    )

    sind_d = nc.dram_tensor("sind3", [3, PI, FR], f32, kind="ExternalInput").ap()
    sims_d = nc.dram_tensor("sims3", [3, PI, FR], f32, kind="ExternalInput").ap()
    attn_d = nc.dram_tensor("attn_pad", [HD, PI, WSEG * K], f32, kind="ExternalInput").ap()
    negc_d = nc.dram_tensor("negc", [128, 1], f32, kind="ExternalInput").ap()
    eps_d = nc.dram_tensor("epsv", [128, 1], f32, kind="ExternalInput").ap()
    out_d = nc.dram_tensor(
        "out", [HD, NSP, ROWS, W, K], f32, kind="ExternalOutput"
    ).ap()

    with tile.TileContext(nc) as tc, ExitStack() as ctx:
        const = ctx.enter_context(tc.tile_pool(name="const", bufs=1))
        work = ctx.enter_context(tc.tile_pool(name="work", bufs=2))
        scr = ctx.enter_context(tc.tile_pool(name="scr", bufs=2))
        outp = ctx.enter_context(tc.tile_pool(name="outp", bufs=4))

        sind_t = [const.tile([PI, FR], f32, name=f"sind{i}") for i in range(3)]
        sims_t = [const.tile([PI, FR], f32, name=f"sims{i}") for i in range(3)]
        for i in range(3):
            nc.sync.dma_start(sind_t[i][:], sind_d[i])
            nc.sync.dma_start(sims_t[i][:], sims_d[i])
        negc_t = const.tile([128, 1], f32)
        nc.sync.dma_start(negc_t[:], negc_d)
        eps_t = const.tile([128, 1], f32)
        nc.sync.dma_start(eps_t[:], eps_d)

        s3 = [t[:].rearrange("p (w s) -> p w s", s=NSP) for t in sind_t]
        w3 = [t[:].rearrange("p (w s) -> p w s", s=NSP) for t in sims_t]

        # exp(attn - c) for all heads up-front (ACT; overlaps the match)
        a_ts = []
        for d in range(HD):
            a_t = work.tile([PI, WSEG * K], f32, name=f"a{d}", bufs=1)
            nc.sync.dma_start(a_t[:], attn_d[d])
            nc.scalar.activation(
                a_t[:], a_t[:], Act.Exp, bias=negc_t[0:PI, :], scale=1.0
            )
            a_ts.append(a_t)

        for wh in range(NWH):
            w0 = wh * WH  # interior w offset of this half
            # ---- match for this w-half: pj[., w, s, k] ----
            pj_t = const.tile([PI, WH * NSP * K], f32, name=f"pj{wh}")  # (w,s,k)
            pj4 = pj_t[:].rearrange("p (w s k) -> p w s k", s=NSP, k=K)
            si = s3[1][:, 1 + w0 : 1 + w0 + WH, :]  # [96, 48, 9]
            for ki, (dh, dw) in enumerate(OFFS):
                pjv = pj4[:, :, :, ki : ki + 1].squeeze(3)  # [96, 48, 9]
                for t in range(NSP):
                    lo = 1 + w0 + dw
                    sjt = s3[dh + 1][:, lo : lo + WH, t : t + 1].broadcast_to(
                        [PI, WH, NSP]
                    )
                    wjt = w3[dh + 1][:, lo : lo + WH, t : t + 1].broadcast_to(
                        [PI, WH, NSP]
                    )
                    eq = scr.tile([PI, FH], f32, tag="eq")
                    e3 = eq[:].rearrange("p (w s) -> p w s", s=NSP)
                    nc.vector.tensor_tensor(e3, si, sjt, Alu.is_equal)
                    if t == 0:
                        nc.vector.tensor_tensor(pjv, e3, wjt, Alu.mult)
                    else:
                        em = scr.tile([PI, FH], f32, tag="em")
                        m3 = em[:].rearrange("p (w s) -> p w s", s=NSP)
                        nc.vector.tensor_tensor(m3, e3, wjt, Alu.mult)
                        nc.vector.tensor_tensor(pjv, pjv, m3, Alu.add)

            # ---- per-head normalize and store for this w-half ----
            pj_v = pj4.transpose([0, 2, 1, 3])  # [96, s, w, k]
            for d in range(HD):
                a_v = (
                    a_ts[d][:]
                    .rearrange("p (w k) -> p w k", k=K)[:, w0 : w0 + WH, :]
                    .unsqueeze(1)
                    .broadcast_to([PI, NSP, WH, K])
                )
                outd = outp.tile([PI, FD], f32, tag="outd")  # (s, w, k)
                o_v = outd[:].rearrange("p (s w k) -> p s w k", s=NSP, k=K)
                den_t = scr.tile([PI, NSP * WH], f32, tag="den")  # (s, w)
                rec_t = scr.tile([PI, NSP * WH], f32, tag="rec")

                # m = a * pj  (GpSimd)
                nc.gpsimd.tensor_tensor(o_v, a_v, pj_v, Alu.mult)
                # den = sum_k m  (VectorE)
                nc.vector.tensor_reduce(
                    den_t[:].rearrange("p (s w) -> p s w", s=NSP),
                    o_v,
                    axis=mybir.AxisListType.X,
                    op=Alu.add,
                )
                # rec = 1/(den+eps) = exp(-ln(den+eps))  (ScalarE)
                nc.scalar.activation(
                    rec_t[:], den_t[:], Act.Ln, bias=eps_t[0:PI, :], scale=1.0
                )
                nc.scalar.activation(rec_t[:], rec_t[:], Act.Exp, scale=-1.0)
                rec_v = (
                    rec_t[:]
                    .rearrange("p (s w) -> p s w", s=NSP)
                    .unsqueeze(3)
                    .broadcast_to([PI, NSP, WH, K])
                )
                # out = m * rec  (GpSimd)
                nc.gpsimd.tensor_tensor(o_v, o_v, rec_v, Alu.mult)

                for g in range(G):
                    src = outd[ROWS * g : ROWS * (g + 1), :].rearrange(
                        "p (s w k) -> p s w k", s=NSP, k=K
                    )
                    wbase = WSEG * g + w0
                    dst = out_d[d, :, :, wbase : wbase + WH, :].transpose(
                        [1, 0, 2, 3]
                    )  # [48, 9, 48, 9]
                    nc.sync.dma_start(dst, src)

    nc.compile()
    return nc


def _get_compiled():
    global _compiled
    if _compiled is None:
        _compiled = _build()
    return _compiled


def _prep_core(attn, sims, sinds, negc, epsv, core):
    b, q = core // 4, core % 4
    h0 = q * ROWS
    cols = np.clip(np.arange(-1, W + 1), 0, W - 1)

    def pad3(x):  # x: [H, W, 9] -> [3, PI, FR]  (dh-shifted, w-padded copies)
        out = np.empty((3, PI, FR), np.float32)
        for i, dh in enumerate((-1, 0, 1)):
            rows = np.clip(np.arange(h0, h0 + ROWS) + dh, 0, H - 1)
            xp = x[rows][:, cols, :]  # [48, 194, 9]
            segs = [xp[:, WSEG * g : WSEG * g + WSPAN, :] for g in range(G)]
            out[i] = np.concatenate(segs, axis=0).reshape(PI, FR)
        return out

    sind3 = pad3(sinds[b])
    sims3 = pad3(sims[b])
    ap = attn[b][:, h0 : h0 + ROWS]  # [HD, 48, 192, 9]
    segs = [ap[:, :, WSEG * g : WSEG * (g + 1), :] for g in range(G)]
    attn_pad = (
        np.concatenate(segs, axis=1).reshape(HD, PI, WSEG * K).astype(np.float32)
    )
    return {
        "sind3": np.ascontiguousarray(sind3),
        "sims3": np.ascontiguousarray(sims3),
        "attn_pad": np.ascontiguousarray(attn_pad),
        "negc": negc,
        "epsv": epsv,
    }


def kernel(attn, sims, sinds, _trace=False):
    attn = np.asarray(attn)
    sims = np.asarray(sims)
    sinds = np.asarray(sinds)

    from concourse import bass_utils

    nc = _get_compiled()

    c = float(np.max(attn))
    negc = np.full((128, 1), -c, dtype=np.float32)
    epsv = np.full((128, 1), EPS, dtype=np.float32)
    in_maps = [
        _prep_core(attn, sims, sinds, negc, epsv, core) for core in range(NCORES)
    ]
    res = bass_utils.run_bass_kernel_spmd(
        nc, in_maps, core_ids=list(range(NCORES)), trace=_trace
    )
    out = np.empty((B, HD, NSP, H, W, K), dtype=np.float32)
    for core in range(NCORES):
        b, q = core // 4, core % 4
        out[b, :, :, ROWS * q : ROWS * (q + 1)] = res.results[core]["out"]
    if _trace:
        return out, res
    return out
